# revision 4
# baseline (speedup 1.0000x reference)
"""Trainium2 Bass kernel for nn_Encoder_Postnet (B=16, T=8192, TP=512, E=256).

Decomposition (exact algebra):
    idx  = aligner_indices(align_phone, text_phone)          # host scan (sequential int walk)
    out  = enc2[b, idx] + PEW[t] + pitch[b,t]*Wp + beats[b,t]*EBd
where
    enc2 = encoder_out @ (I + W_pos)            # tiny per-batch matmul, done on device (PE)
    PEW  = pe @ W_pos + b_pos + b_pitch + emb_beats[0]       # host (constant table)
    Wp   = W_pitch[0],  EBd = emb_beats[1] - emb_beats[0]

Sharding: pure data parallel, 2 batches per core across 8 cores.
Device per core: PE transposes enc + computes enc2 -> DRAM scratch; dma_gather
pulls enc2 rows per frame; PE computes pitch/beats rank-2 term into PSUM;
two DVE tensor_tensor adds combine gather + PEW + PSUM; DMA out.
"""
import numpy as np
import ml_dtypes

import concourse.bacc as bacc
import concourse.bass as bass
import concourse.mybir as mybir
import concourse.tile as tile
from concourse import library_config
from concourse.bass_utils import run_bass_kernel_spmd

# ---- problem constants (hardcoded per harness contract) ----
B, T, TP, E = 16, 8192, 512, 256
NCORES = 8
BPC = B // NCORES            # batches per core = 2
ROWS = BPC * TP              # enc rows per core = 1024
CH = 1024                    # frames per gather chunk
NCH = T // CH                # chunks per batch = 8
NG = CH // 128               # 128-frame groups per chunk = 8

F32 = mybir.dt.float32
BF16 = mybir.dt.bfloat16
I16 = mybir.dt.int16


# ---------------- host-side pieces ----------------

def aligner_idx_host(align_phone: np.ndarray, text_phone: np.ndarray) -> np.ndarray:
    """Exact numpy equivalent of the reference aligner_indices scan."""
    b, t = align_phone.shape
    tp_last = text_phone.shape[1] - 1
    idx = np.zeros((b, t), dtype=np.int32)
    ind = np.zeros(b, dtype=np.int32)
    before = text_phone[:, 0].copy()
    barange = np.arange(b)
    for j in range(1, t):
        a = align_phone[:, j]
        same = a == before
        ind = np.minimum(np.where(same, ind, ind + 1), tp_last)
        before = np.where(same, before, text_phone[barange, ind])
        idx[:, j] = ind
    return idx


def sinusoid_pe_host(length, dim):
    pos = np.arange(length, dtype=np.float32)[:, None]
    div = np.exp(np.arange(0, dim, 2, dtype=np.float32) * (-(np.log(10000.0) / dim)))
    ang = pos * div
    pe = np.zeros((length, dim), np.float32)
    pe[:, 0::2] = np.sin(ang)
    pe[:, 1::2] = np.cos(ang)
    return pe


def pack_idx16(idx_rows: np.ndarray) -> np.ndarray:
    """idx_rows [BPC, T] int (already offset by local_b*TP) ->
    [128, BPC, NCH, CH//16] int16 in dma_gather's wrapped layout:
    within a chunk, index j lives at partition j%16, col j//16, replicated 8x."""
    out = np.empty((128, BPC, NCH, CH // 16), dtype=np.int16)
    for lb in range(BPC):
        for c in range(NCH):
            blk = idx_rows[lb, c * CH:(c + 1) * CH].reshape(CH // 16, 16).T  # [16, CH//16]
            out[:, lb, c, :] = np.tile(blk.astype(np.int16), (8, 1))
    return out


# ---------------- device program ----------------

def build_program() -> bass.Bass:
    nc = bacc.Bacc("TRN2", num_devices=NCORES, debug=False, enable_asserts=False)
    enc = nc.dram_tensor("enc", [ROWS, E], F32, kind="ExternalInput")
    w2 = nc.dram_tensor("w2", [E, E], F32, kind="ExternalInput")
    ident = nc.dram_tensor("ident", [128, 128], F32, kind="ExternalInput")
    w3 = nc.dram_tensor("w3", [2, E], BF16, kind="ExternalInput")
    pew = nc.dram_tensor("pew", [T, E], F32, kind="ExternalInput")
    aux = nc.dram_tensor("aux", [2, BPC * T], BF16, kind="ExternalInput")
    idx16 = nc.dram_tensor("idx16", [128, BPC, NCH, CH // 16], I16, kind="ExternalInput")
    out = nc.dram_tensor("out", [BPC * T, E], F32, kind="ExternalOutput")

    with tile.TileContext(nc) as tc:
        with (
            tc.tile_pool(name="const", bufs=1) as cpool,
            tc.tile_pool(name="dram", bufs=1, space="DRAM") as dpool,
            tc.tile_pool(name="work", bufs=3) as wpool,
            tc.tile_pool(name="gather", bufs=3) as gpool,
            tc.tile_pool(name="outp", bufs=3) as opool,
        ):
            nc.gpsimd.load_library(library_config.mlp)

            # ---- constants ----
            pew_sb = cpool.tile([128, T // 128, E], F32, tag="pew")
            nc.sync.dma_start(pew_sb[:], pew.ap().rearrange("(c p) e -> p c e", p=128))
            w2_sb = cpool.tile([128, 2, E], F32, tag="w2")
            nc.sync.dma_start(w2_sb[:], w2.ap().rearrange("(k p) e -> p k e", p=128))
            ident_sb = cpool.tile([128, 128], F32, tag="ident")
            nc.sync.dma_start(ident_sb[:], ident.ap())
            w3_sb = cpool.tile([2, E], BF16, tag="w3")
            nc.sync.dma_start(w3_sb[:], w3.ap())
            aux_sb = cpool.tile([2, BPC * T], BF16, tag="aux")
            nc.sync.dma_start(aux_sb[:], aux.ap())
            idx_sb = cpool.tile([128, BPC, NCH, CH // 16], I16, tag="idx")
            nc.sync.dma_start(idx_sb[:], idx16.ap())

            # ---- prologue: enc2 = enc @ (I + W_pos) -> DRAM scratch ----
            enc_sb = cpool.tile([128, ROWS // 128, E], F32, tag="enc")
            nc.sync.dma_start(enc_sb[:], enc.ap().rearrange("(r p) e -> p r e", p=128))
            encT_sb = cpool.tile([128, 2 * (ROWS // 128), 128], F32, tag="encT")
            enc2_dram = dpool.tile([ROWS, E], F32, tag="enc2")
            with tc.tile_pool(name="psum_pro", bufs=2, space="PSUM") as ppro:
                for rt in range(ROWS // 128):
                    for k in range(2):
                        pt = ppro.tile([128, 128], F32, tag="ptr")
                        nc.tensor.transpose(
                            out=pt[:],
                            in_=enc_sb[:, rt, k * 128:(k + 1) * 128],
                            identity=ident_sb[:],
                        )
                        nc.vector.tensor_copy(
                            out=encT_sb[:, k * (ROWS // 128) + rt, :], in_=pt[:]
                        )
                for rt in range(ROWS // 128):
                    pe2 = ppro.tile([128, E], F32, tag="pe2")
                    nc.tensor.matmul(
                        out=pe2[:], lhsT=encT_sb[:, rt, :], rhs=w2_sb[:, 0, :],
                        start=True, stop=False,
                    )
                    nc.tensor.matmul(
                        out=pe2[:], lhsT=encT_sb[:, (ROWS // 128) + rt, :],
                        rhs=w2_sb[:, 1, :], start=False, stop=True,
                    )
                    e2 = wpool.tile([128, E], F32, tag="e2")
                    nc.vector.tensor_copy(out=e2[:], in_=pe2[:])
                    nc.sync.dma_start(enc2_dram[rt * 128:(rt + 1) * 128, :], e2[:])

            # ---- main loop ----
            with tc.tile_pool(name="psum_aux", bufs=2, space="PSUM") as paux:
                for lb in range(BPC):
                    for c in range(NCH):
                        g = gpool.tile([128, NG, E], F32, tag="g")
                        nc.gpsimd.dma_gather(
                            g[:], enc2_dram[:], idx_sb[:, lb, c, :], CH, CH, E,
                        )
                        ps = paux.tile([128, NG, E], F32, tag="ps")
                        for grp in range(NG):
                            t0 = lb * T + c * CH + grp * 128
                            nc.tensor.matmul(
                                out=ps[:, grp, :],
                                lhsT=aux_sb[:, t0:t0 + 128],
                                rhs=w3_sb[:],
                                start=True, stop=True,
                            )
                        o = opool.tile([128, NG, E], F32, tag="o")
                        nc.vector.tensor_tensor(
                            out=o[:], in0=g[:],
                            in1=pew_sb[:, c * NG:(c + 1) * NG, :],
                            op=mybir.AluOpType.add,
                        )
                        nc.vector.tensor_tensor(
                            out=o[:], in0=o[:], in1=ps[:],
                            op=mybir.AluOpType.add,
                        )
                        base = lb * T + c * CH
                        nc.sync.dma_start(
                            out.ap()[base:base + CH, :].rearrange(
                                "(cc p) e -> p cc e", p=128
                            ),
                            o[:],
                        )
    nc.compile()
    return nc


_PROGRAM_CACHE: list = []


def get_program() -> bass.Bass:
    if not _PROGRAM_CACHE:
        _PROGRAM_CACHE.append(build_program())
    return _PROGRAM_CACHE[0]


# ---------------- host orchestration ----------------

def make_in_maps(encoder_out, align_phone, text_phone, pitch, beats,
                 W_pitch, b_pitch, W_pos, b_pos, emb_beats):
    idx = aligner_idx_host(np.asarray(align_phone), np.asarray(text_phone))  # [B, T]

    pe = sinusoid_pe_host(T, E)
    pew = (pe @ np.asarray(W_pos) + np.asarray(b_pos) + np.asarray(b_pitch)
           + np.asarray(emb_beats)[0]).astype(np.float32)
    w2 = (np.eye(E, dtype=np.float32) + np.asarray(W_pos)).astype(np.float32)
    ident = np.eye(128, dtype=np.float32)
    wp = np.asarray(W_pitch)[0].astype(np.float32)
    ebd = (np.asarray(emb_beats)[1] - np.asarray(emb_beats)[0]).astype(np.float32)
    w3 = np.stack([wp, ebd]).astype(ml_dtypes.bfloat16)  # [2, E]

    enc = np.ascontiguousarray(np.asarray(encoder_out), dtype=np.float32)  # [B, TP, E]
    pitch2 = np.asarray(pitch)[:, :, 0].astype(np.float32)
    beats2 = np.asarray(beats)[:, :, 0].astype(np.float32)

    in_maps = []
    for core in range(NCORES):
        bs = slice(core * BPC, (core + 1) * BPC)
        idx_rows = idx[bs] + (np.arange(BPC)[:, None] * TP)  # local row offsets
        aux = np.stack([pitch2[bs].reshape(-1), beats2[bs].reshape(-1)])  # [2, BPC*T]
        in_maps.append({
            "enc": enc[bs].reshape(ROWS, E),
            "w2": w2,
            "ident": ident,
            "w3": w3,
            "pew": pew,
            "aux": aux.astype(ml_dtypes.bfloat16),
            "idx16": pack_idx16(idx_rows),
        })
    return in_maps


def kernel(**inputs) -> np.ndarray:
    nc = get_program()
    in_maps = make_in_maps(**inputs)
    res = run_bass_kernel_spmd(nc, in_maps, core_ids=list(range(NCORES)))
    outs = [r["out"].reshape(BPC, T, E) for r in res.results]
    return np.concatenate(outs, axis=0).astype(np.float32)


# revision 8
# speedup vs baseline: 1.4802x; 1.4802x over previous
"""Trainium2 Bass kernel for nn_Encoder_Postnet (B=16, T=8192, TP=512, E=256).

Exact algebra:
    idx  = aligner_indices(align_phone, text_phone)     # host scan (sequential int walk)
    out  = enc2[b, idx] + PEW[t] + pitch[b,t]*Wp + beats[b,t]*EBd
where
    enc2 = encoder_out @ (I + W_pos)                    # device PE, f32
    PEW  = pe @ W_pos + b_pos + b_pitch + emb_beats[0]  # host constant table
    Wp   = W_pitch[0],  EBd = emb_beats[1] - emb_beats[0]

Sharding: pure data parallel, 2 batches per core across 8 cores.

The frame gather enc2[idx] exploits idx monotonicity: each 128-frame group's
rows fall in only 1-2 aligned 128-row blocks of enc2, so the gather becomes a
one-hot matmul on the (otherwise idle) TensorE: psum_g = onehotT_g.T @
enc2_block, with the one-hot matrices built host-side (host knows idx) and
DMA'd in as fp16. The rank-2 pitch/beats term accumulates into the same PSUM
via K=2 matmuls, and a single DVE tensor_tensor adds the resident PEW table
and writes fp16 output. Works for arbitrary idx (any number of blocks per
group — the host plan just emits more matmuls).
"""
import numpy as np

import concourse.bacc as bacc
import concourse.bass as bass
import concourse.mybir as mybir
import concourse.tile as tile
from concourse.bass_utils import run_bass_kernel_spmd

# ---- problem constants (hardcoded per harness contract) ----
B, T, TP, E = 16, 8192, 512, 256
NCORES = 8
BPC = B // NCORES            # batches per core = 2
ROWS = BPC * TP              # enc2 rows per core = 1024
NBLK = ROWS // 128           # 128-row blocks of enc2 = 8
CH = 1024                    # frames per chunk
NCH = T // CH                # chunks per batch = 8
NG = CH // 128               # 128-frame groups per chunk = 8

F32 = mybir.dt.float32
FP16 = mybir.dt.float16

_PROGRAM_CACHE: dict = {}


# ---------------- host-side pieces ----------------

def aligner_idx_host(align_phone: np.ndarray, text_phone: np.ndarray) -> np.ndarray:
    """Exact numpy equivalent of the reference aligner_indices scan."""
    b, t = align_phone.shape
    tp_last = text_phone.shape[1] - 1
    idx = np.zeros((b, t), dtype=np.int32)
    ind = np.zeros(b, dtype=np.int32)
    before = text_phone[:, 0].copy()
    barange = np.arange(b)
    for j in range(1, t):
        a = align_phone[:, j]
        same = a == before
        ind = np.minimum(np.where(same, ind, ind + 1), tp_last)
        before = np.where(same, before, text_phone[barange, ind])
        idx[:, j] = ind
    return idx


def sinusoid_pe_host(length, dim):
    pos = np.arange(length, dtype=np.float32)[:, None]
    div = np.exp(np.arange(0, dim, 2, dtype=np.float32) * (-(np.log(10000.0) / dim)))
    ang = pos * div
    pe = np.zeros((length, dim), np.float32)
    pe[:, 0::2] = np.sin(ang)
    pe[:, 1::2] = np.cos(ang)
    return pe


def group_blocks(idx_rows: np.ndarray):
    """idx_rows: [BPC, T] local enc2 row per frame (already offset by lb*TP).
    Returns blocks[chunk_index][group] = sorted list of 128-row blocks the
    group's rows touch."""
    out = []
    for lb in range(BPC):
        for c in range(NCH):
            chunk = []
            for g in range(NG):
                f0 = c * CH + g * 128
                gi = idx_rows[lb, f0:f0 + 128]
                chunk.append(sorted(int(x) for x in np.unique(gi >> 7)))
            out.append(chunk)
    return out


# ---------------- device program ----------------

def build_program(plans) -> bass.Bass:
    """plans: per-core list of (plan, ncols)."""
    nc = bacc.Bacc("TRN2", num_devices=NCORES, debug=False, enable_asserts=False)
    ncols_max = max(p[1] for p in plans)

    enc = nc.dram_tensor("enc", [ROWS, E], F32, kind="ExternalInput")
    w2 = nc.dram_tensor("w2", [E, E], F32, kind="ExternalInput")
    ident = nc.dram_tensor("ident", [128, 128], F32, kind="ExternalInput")
    w3 = nc.dram_tensor("w3", [2, E], FP16, kind="ExternalInput")
    pew = nc.dram_tensor("pew", [T, E], F32, kind="ExternalInput")
    aux = nc.dram_tensor("aux", [2, BPC * T], FP16, kind="ExternalInput")
    oh = nc.dram_tensor("oh", [128, ncols_max], FP16, kind="ExternalInput")
    out = nc.dram_tensor("out", [BPC * T, E], FP16, kind="ExternalOutput")

    # SPMD note: all cores run the same program; use core 0's plan shape as
    # canonical. We require all cores to share one plan (see make_in_maps:
    # plans are padded/unified).
    plan = plans[0][0]

    with tile.TileContext(nc) as tc:
        with (
            tc.tile_pool(name="const", bufs=1) as cpool,
            tc.tile_pool(name="work", bufs=3) as wpool,
            tc.tile_pool(name="outp", bufs=3) as opool,
        ):
            # ---- constants ----
            pew_sb = cpool.tile([128, T // 128, E], F32, tag="pew")
            nc.sync.dma_start(pew_sb[:], pew.ap().rearrange("(c p) e -> p c e", p=128))
            w2_sb = cpool.tile([128, 2, E], F32, tag="w2")
            nc.sync.dma_start(w2_sb[:], w2.ap().rearrange("(k p) e -> p k e", p=128))
            ident_sb = cpool.tile([128, 128], F32, tag="ident")
            nc.sync.dma_start(ident_sb[:], ident.ap())
            w3_sb = cpool.tile([2, E], FP16, tag="w3")
            nc.sync.dma_start(w3_sb[:], w3.ap())
            aux_sb = cpool.tile([2, BPC * T], FP16, tag="aux")
            nc.sync.dma_start(aux_sb[:], aux.ap())
            oh_sb = cpool.tile([128, ncols_max], FP16, tag="oh")
            nc.sync.dma_start(oh_sb[:], oh.ap())

            # ---- prologue: enc2 = enc @ (I + W_pos) -> fp16 in SBUF ----
            enc_sb = cpool.tile([128, NBLK, E], F32, tag="enc")
            nc.sync.dma_start(enc_sb[:], enc.ap().rearrange("(r p) e -> p r e", p=128))
            encT_sb = cpool.tile([128, 2 * NBLK, 128], F32, tag="encT")
            enc2_sb = cpool.tile([128, NBLK, E], FP16, tag="enc2")
            with tc.tile_pool(name="psum_pro", bufs=4, space="PSUM") as ppro:
                for rt in range(NBLK):
                    for k in range(2):
                        pt = ppro.tile([128, 128], F32, tag="ptr")
                        nc.tensor.transpose(
                            out=pt[:],
                            in_=enc_sb[:, rt, k * 128:(k + 1) * 128],
                            identity=ident_sb[:],
                        )
                        nc.vector.tensor_copy(
                            out=encT_sb[:, k * NBLK + rt, :], in_=pt[:]
                        )
                for rt in range(NBLK):
                    pe2 = ppro.tile([128, E], F32, tag="pe2")
                    nc.tensor.matmul(
                        out=pe2[:], lhsT=encT_sb[:, rt, :], rhs=w2_sb[:, 0, :],
                        start=True, stop=False,
                    )
                    nc.tensor.matmul(
                        out=pe2[:], lhsT=encT_sb[:, NBLK + rt, :],
                        rhs=w2_sb[:, 1, :], start=False, stop=True,
                    )
                    nc.vector.tensor_copy(out=enc2_sb[:, rt, :], in_=pe2[:])

            # ---- main loop ----
            with tc.tile_pool(name="psum_main", bufs=2, space="PSUM") as pmain:
                for lb in range(BPC):
                    for c in range(NCH):
                        chunk_plan = plan[lb * NCH + c]
                        ps = pmain.tile([128, NG, E], F32, tag="ps")
                        for g in range(NG):
                            entries = chunk_plan[g]
                            for j, (coloff, blk) in enumerate(entries):
                                nc.tensor.matmul(
                                    out=ps[:, g, :],
                                    lhsT=oh_sb[:, coloff:coloff + 128],
                                    rhs=enc2_sb[:, blk, :],
                                    start=(j == 0), stop=False,
                                )
                            t0 = lb * T + c * CH + g * 128
                            nc.tensor.matmul(
                                out=ps[:, g, :],
                                lhsT=aux_sb[:, t0:t0 + 128],
                                rhs=w3_sb[:],
                                start=False, stop=True,
                            )
                        o = opool.tile([128, NG, E], FP16, tag="o")
                        nc.vector.tensor_tensor(
                            out=o[:], in0=ps[:],
                            in1=pew_sb[:, c * NG:(c + 1) * NG, :],
                            op=mybir.AluOpType.add,
                        )
                        base = lb * T + c * CH
                        nc.sync.dma_start(
                            out.ap()[base:base + CH, :].rearrange(
                                "(cc p) e -> p cc e", p=128
                            ),
                            o[:],
                        )
    nc.compile()
    return nc


# ---------------- host orchestration ----------------

def make_in_maps(encoder_out, align_phone, text_phone, pitch, beats,
                 W_pitch, b_pitch, W_pos, b_pos, emb_beats):
    idx = aligner_idx_host(np.asarray(align_phone), np.asarray(text_phone))  # [B, T]

    pe = sinusoid_pe_host(T, E)
    pew = (pe @ np.asarray(W_pos) + np.asarray(b_pos) + np.asarray(b_pitch)
           + np.asarray(emb_beats)[0]).astype(np.float32)
    w2 = (np.eye(E, dtype=np.float32) + np.asarray(W_pos)).astype(np.float32)
    ident = np.eye(128, dtype=np.float32)
    wp = np.asarray(W_pitch)[0].astype(np.float32)
    ebd = (np.asarray(emb_beats)[1] - np.asarray(emb_beats)[0]).astype(np.float32)
    w3 = np.stack([wp, ebd]).astype(np.float16)  # [2, E]

    enc = np.ascontiguousarray(np.asarray(encoder_out), dtype=np.float32)  # [B, TP, E]
    pitch2 = np.asarray(pitch)[:, :, 0].astype(np.float32)
    beats2 = np.asarray(beats)[:, :, 0].astype(np.float32)

    # per-core local row indices + per-group block lists
    idx_rows_all = []
    blocks_all = []
    for core in range(NCORES):
        bs = slice(core * BPC, (core + 1) * BPC)
        idx_rows = idx[bs] + (np.arange(BPC)[:, None] * TP)
        idx_rows_all.append(idx_rows)
        blocks_all.append(group_blocks(idx_rows))

    # SPMD requires one program: canonical plan = per (chunk, group) union of
    # blocks across cores; column offsets assigned in order.
    canon_plan = []
    off = 0
    for ci in range(BPC * NCH):
        chunk_plan = []
        for g in range(NG):
            blocks = sorted({b for core in range(NCORES)
                             for b in blocks_all[core][ci][g]})
            entries = []
            for blk in blocks:
                entries.append((off, blk))
                off += 128
            chunk_plan.append(entries)
        canon_plan.append(chunk_plan)
    ncols_total = off

    per_core = []
    m = np.arange(128)
    for core in range(NCORES):
        bs = slice(core * BPC, (core + 1) * BPC)
        idx_rows = idx_rows_all[core]
        onehot = np.zeros((128, ncols_total), dtype=np.float16)
        for ci in range(BPC * NCH):
            lb, c = divmod(ci, NCH)
            for g in range(NG):
                f0 = c * CH + g * 128
                gi = idx_rows[lb, f0:f0 + 128]
                for (coloff, blk) in canon_plan[ci][g]:
                    p = gi - blk * 128
                    sel = (p >= 0) & (p < 128)
                    onehot[p[sel], coloff + m[sel]] = 1.0
        aux_arr = np.stack([pitch2[bs].reshape(-1), beats2[bs].reshape(-1)])
        per_core.append({
            "enc": enc[bs].reshape(ROWS, E),
            "w2": w2,
            "ident": ident,
            "w3": w3,
            "pew": pew,
            "aux": aux_arr.astype(np.float16),
            "oh": onehot,
        })

    return per_core, canon_plan, ncols_total


def get_program(canon_plan, ncols_total) -> bass.Bass:
    key = (tuple(tuple(tuple(e) for e in cp) for cp in canon_plan), ncols_total)
    if key not in _PROGRAM_CACHE:
        _PROGRAM_CACHE[key] = build_program([(canon_plan, ncols_total)] * NCORES)
    return _PROGRAM_CACHE[key]


def kernel(**inputs) -> np.ndarray:
    in_maps, canon_plan, ncols_total = make_in_maps(**inputs)
    nc = get_program(canon_plan, ncols_total)
    res = run_bass_kernel_spmd(nc, in_maps, core_ids=list(range(NCORES)))
    outs = [r["out"].astype(np.float32).reshape(BPC, T, E) for r in res.results]
    return np.concatenate(outs, axis=0)


# revision 10
# speedup vs baseline: 1.9621x; 1.3256x over previous
"""Trainium2 Bass kernel for nn_Encoder_Postnet (B=16, T=8192, TP=512, E=256).

Exact algebra:
    idx  = aligner_indices(align_phone, text_phone)     # host scan (sequential int walk)
    out  = enc2[b, idx] + PEW[t] + pitch[b,t]*Wp + beats[b,t]*EBd
where
    enc2 = encoder_out @ (I + W_pos)                    # device PE, f32
    PEW  = pe @ W_pos + b_pos + b_pitch + emb_beats[0]  # host constant table
    Wp   = W_pitch[0],  EBd = emb_beats[1] - emb_beats[0]

Sharding: pure data parallel, 2 batches per core across 8 cores.

The frame gather enc2[idx] exploits idx monotonicity: a 128-frame group's rows
span few enc2 rows, so the gather becomes one matmul per group on the
(otherwise idle) TensorE against a 64-aligned "window" of enc2: the device
keeps enc2win[k] = [enc2 rows 64k..64k+126; Wp; EBd] (128 partitions), and the
host builds lhsT columns: rows 0-125 one-hot selecting the window row per
frame, rows 126-127 carrying pitch/beats so the rank-2 aux term rides in the
SAME matmul. PSUM accumulates; one DVE tensor_tensor adds the resident PEW
table and writes fp16 output. Arbitrary idx is handled by covering a group
with multiple windows (extra accumulating matmuls); SPMD uniformity by taking
the cross-core union of window entries (unused entries have all-zero lhsT
columns).
"""
import numpy as np

import concourse.bacc as bacc
import concourse.bass as bass
import concourse.mybir as mybir
import concourse.tile as tile
from concourse.bass_utils import run_bass_kernel_spmd

# ---- problem constants (hardcoded per harness contract) ----
B, T, TP, E = 16, 8192, 512, 256
NCORES = 8
BPC = B // NCORES            # batches per core = 2
ROWS = BPC * TP              # enc2 rows per core = 1024
NBLK = ROWS // 128           # 128-row blocks of enc2 = 8
NWIN = ROWS // 64            # 64-aligned windows = 16
CH = 1024                    # frames per chunk
NCH = T // CH                # chunks per batch = 8
NG = CH // 128               # 128-frame groups per chunk = 8
NCHUNK = BPC * NCH           # chunks per core = 16
WROWS = 126                  # usable enc2 rows per window (126/127 = Wp/EBd)

F32 = mybir.dt.float32
FP16 = mybir.dt.float16

_PROGRAM_CACHE: dict = {}


# ---------------- host-side pieces ----------------

def aligner_idx_host(align_phone: np.ndarray, text_phone: np.ndarray) -> np.ndarray:
    """Exact numpy equivalent of the reference aligner_indices scan."""
    b, t = align_phone.shape
    tp_last = text_phone.shape[1] - 1
    idx = np.zeros((b, t), dtype=np.int32)
    ind = np.zeros(b, dtype=np.int32)
    before = text_phone[:, 0].copy()
    barange = np.arange(b)
    for j in range(1, t):
        a = align_phone[:, j]
        same = a == before
        ind = np.minimum(np.where(same, ind, ind + 1), tp_last)
        before = np.where(same, before, text_phone[barange, ind])
        idx[:, j] = ind
    return idx


def sinusoid_pe_host(length, dim):
    pos = np.arange(length, dtype=np.float32)[:, None]
    div = np.exp(np.arange(0, dim, 2, dtype=np.float32) * (-(np.log(10000.0) / dim)))
    ang = pos * div
    pe = np.zeros((length, dim), np.float32)
    pe[:, 0::2] = np.sin(ang)
    pe[:, 1::2] = np.cos(ang)
    return pe


def windows_for_group(gi: np.ndarray) -> list:
    """Minimal 64-aligned 126-row windows covering the rows in gi (sorted)."""
    rows = np.unique(gi)
    wins = []
    i = 0
    while i < len(rows):
        k = int(rows[i]) // 64
        wins.append(k)
        top = 64 * k + WROWS  # rows [64k, 64k+126) covered
        while i < len(rows) and rows[i] < top:
            i += 1
    return wins


def group_windows(idx_rows: np.ndarray):
    """per chunk per group: list of window ids for this core."""
    out = []
    for lb in range(BPC):
        for c in range(NCH):
            chunk = []
            for g in range(NG):
                f0 = c * CH + g * 128
                chunk.append(windows_for_group(idx_rows[lb, f0:f0 + 128]))
            out.append(chunk)
    return out


# ---------------- device program ----------------

def build_program(canon_plan, ncols_total) -> bass.Bass:
    """canon_plan[ci][g] = list of (coloff, win_k)."""
    nc = bacc.Bacc("TRN2", num_devices=NCORES, debug=False, enable_asserts=False)

    enc = nc.dram_tensor("enc", [ROWS, E], F32, kind="ExternalInput")
    w2 = nc.dram_tensor("w2", [E, E], F32, kind="ExternalInput")
    ident = nc.dram_tensor("ident", [128, 128], F32, kind="ExternalInput")
    w3 = nc.dram_tensor("w3", [2, E], FP16, kind="ExternalInput")
    pew = nc.dram_tensor("pew", [T, E], F32, kind="ExternalInput")
    oh = nc.dram_tensor("oh", [128, ncols_total], FP16, kind="ExternalInput")
    out = nc.dram_tensor("out", [BPC * T, E], FP16, kind="ExternalOutput")

    with tile.TileContext(nc) as tc:
        with (
            tc.tile_pool(name="const", bufs=1) as cpool,
            tc.tile_pool(name="outp", bufs=3) as opool,
        ):
            # ---- constants ----
            pew_sb = cpool.tile([128, T // 128, E], F32, tag="pew")
            nc.sync.dma_start(pew_sb[:], pew.ap().rearrange("(c p) e -> p c e", p=128))
            w2_sb = cpool.tile([128, 2, E], F32, tag="w2")
            nc.sync.dma_start(w2_sb[:], w2.ap().rearrange("(k p) e -> p k e", p=128))
            ident_sb = cpool.tile([128, 128], F32, tag="ident")
            nc.sync.dma_start(ident_sb[:], ident.ap())
            oh_sb = cpool.tile([128, ncols_total], FP16, tag="oh")
            nc.sync.dma_start(oh_sb[:], oh.ap())

            # ---- prologue: enc2 = enc @ (I + W_pos), fp16, then windows ----
            enc_sb = cpool.tile([128, NBLK, E], F32, tag="enc")
            nc.sync.dma_start(enc_sb[:], enc.ap().rearrange("(r p) e -> p r e", p=128))
            encT_sb = cpool.tile([128, 2 * NBLK, 128], F32, tag="encT")
            enc2_sb = cpool.tile([128, NBLK, E], FP16, tag="enc2")
            win_sb = cpool.tile([128, NWIN, E], FP16, tag="win")
            nc.gpsimd.memset(win_sb[:], 0.0)
            with tc.tile_pool(name="psum_pro", bufs=4, space="PSUM") as ppro:
                for rt in range(NBLK):
                    for k in range(2):
                        pt = ppro.tile([128, 128], F32, tag="ptr")
                        nc.tensor.transpose(
                            out=pt[:],
                            in_=enc_sb[:, rt, k * 128:(k + 1) * 128],
                            identity=ident_sb[:],
                        )
                        nc.vector.tensor_copy(
                            out=encT_sb[:, k * NBLK + rt, :], in_=pt[:]
                        )
                for rt in range(NBLK):
                    pe2 = ppro.tile([128, E], F32, tag="pe2")
                    nc.tensor.matmul(
                        out=pe2[:], lhsT=encT_sb[:, rt, :], rhs=w2_sb[:, 0, :],
                        start=True, stop=False,
                    )
                    nc.tensor.matmul(
                        out=pe2[:], lhsT=encT_sb[:, NBLK + rt, :],
                        rhs=w2_sb[:, 1, :], start=False, stop=True,
                    )
                    nc.vector.tensor_copy(out=enc2_sb[:, rt, :], in_=pe2[:])
            # windows: even k = aligned block copy; odd k = two partition-shifted pieces
            for k in range(NWIN):
                if k % 2 == 0:
                    nc.sync.dma_start(win_sb[0:126, k, :], enc2_sb[0:126, k // 2, :])
                else:
                    nc.sync.dma_start(win_sb[0:64, k, :], enc2_sb[64:128, k // 2, :])
                    if k // 2 + 1 < NBLK:
                        nc.sync.dma_start(
                            win_sb[64:126, k, :], enc2_sb[0:62, k // 2 + 1, :]
                        )
                nc.sync.dma_start(win_sb[126:128, k, :], w3.ap())

            # ---- main loop ----
            with tc.tile_pool(name="psum_main", bufs=2, space="PSUM") as pmain:
                for ci in range(NCHUNK):
                    lb, c = divmod(ci, NCH)
                    ps = pmain.tile([128, NG, E], F32, tag="ps")
                    for g in range(NG):
                        entries = canon_plan[ci][g]
                        n = len(entries)
                        for j, (coloff, k) in enumerate(entries):
                            nc.tensor.matmul(
                                out=ps[:, g, :],
                                lhsT=oh_sb[:, coloff:coloff + 128],
                                rhs=win_sb[:, k, :],
                                start=(j == 0), stop=(j == n - 1),
                            )
                    o = opool.tile([128, NG, E], FP16, tag="o")
                    nc.vector.tensor_tensor(
                        out=o[:], in0=ps[:],
                        in1=pew_sb[:, c * NG:(c + 1) * NG, :],
                        op=mybir.AluOpType.add,
                    )
                    base = lb * T + c * CH
                    nc.sync.dma_start(
                        out.ap()[base:base + CH, :].rearrange(
                            "(cc p) e -> p cc e", p=128
                        ),
                        o[:],
                    )
    nc.compile()
    return nc


# ---------------- host orchestration ----------------

def make_in_maps(encoder_out, align_phone, text_phone, pitch, beats,
                 W_pitch, b_pitch, W_pos, b_pos, emb_beats):
    idx = aligner_idx_host(np.asarray(align_phone), np.asarray(text_phone))  # [B, T]

    pe = sinusoid_pe_host(T, E)
    pew = (pe @ np.asarray(W_pos) + np.asarray(b_pos) + np.asarray(b_pitch)
           + np.asarray(emb_beats)[0]).astype(np.float32)
    w2 = (np.eye(E, dtype=np.float32) + np.asarray(W_pos)).astype(np.float32)
    ident = np.eye(128, dtype=np.float32)
    wp = np.asarray(W_pitch)[0].astype(np.float32)
    ebd = (np.asarray(emb_beats)[1] - np.asarray(emb_beats)[0]).astype(np.float32)
    w3 = np.stack([wp, ebd]).astype(np.float16)  # [2, E]

    enc = np.ascontiguousarray(np.asarray(encoder_out), dtype=np.float32)  # [B, TP, E]
    pitch2 = np.asarray(pitch)[:, :, 0].astype(np.float32)
    beats2 = np.asarray(beats)[:, :, 0].astype(np.float32)

    idx_rows_all = []
    wins_all = []
    for core in range(NCORES):
        bs = slice(core * BPC, (core + 1) * BPC)
        idx_rows = idx[bs] + (np.arange(BPC)[:, None] * TP)
        idx_rows_all.append(idx_rows)
        wins_all.append(group_windows(idx_rows))

    # canonical plan: per (chunk, group) union of window ids across cores
    canon_plan = []
    off = 0
    for ci in range(NCHUNK):
        chunk_plan = []
        for g in range(NG):
            ks = sorted({k for core in range(NCORES) for k in wins_all[core][ci][g]})
            entries = []
            for k in ks:
                entries.append((off, k))
                off += 128
            chunk_plan.append(entries)
        canon_plan.append(chunk_plan)
    ncols_total = off

    per_core = []
    m = np.arange(128)
    for core in range(NCORES):
        bs = slice(core * BPC, (core + 1) * BPC)
        idx_rows = idx_rows_all[core]
        onehot = np.zeros((128, ncols_total), dtype=np.float16)
        for ci in range(NCHUNK):
            lb, c = divmod(ci, NCH)
            for g in range(NG):
                f0 = c * CH + g * 128
                gi = idx_rows[lb, f0:f0 + 128]
                my_wins = wins_all[core][ci][g]
                entries = canon_plan[ci][g]
                # row -> my window (first of my windows covering it)
                assigned = np.full(128, -1, dtype=np.int64)
                for k in my_wins:
                    in_win = (gi >= 64 * k) & (gi < 64 * k + WROWS) & (assigned < 0)
                    assigned[in_win] = k
                aux_done = False
                for (coloff, k) in entries:
                    if k not in my_wins:
                        continue
                    sel = assigned == k
                    onehot[gi[sel] - 64 * k, coloff + m[sel]] = 1.0
                    if not aux_done:
                        fr = slice(c * CH + g * 128, c * CH + g * 128 + 128)
                        onehot[126, coloff:coloff + 128] = pitch2[core * BPC + lb, fr]
                        onehot[127, coloff:coloff + 128] = beats2[core * BPC + lb, fr]
                        aux_done = True
        per_core.append({
            "enc": enc[bs].reshape(ROWS, E),
            "w2": w2,
            "ident": ident,
            "w3": w3,
            "pew": pew,
            "oh": onehot,
        })

    return per_core, canon_plan, ncols_total


def get_program(canon_plan, ncols_total) -> bass.Bass:
    key = (tuple(tuple(tuple(e) for e in cg) for cg in canon_plan), ncols_total)
    if key not in _PROGRAM_CACHE:
        _PROGRAM_CACHE[key] = build_program(canon_plan, ncols_total)
    return _PROGRAM_CACHE[key]


def kernel(**inputs) -> np.ndarray:
    in_maps, canon_plan, ncols_total = make_in_maps(**inputs)
    nc = get_program(canon_plan, ncols_total)
    res = run_bass_kernel_spmd(nc, in_maps, core_ids=list(range(NCORES)))
    outs = [r["out"].astype(np.float32).reshape(BPC, T, E) for r in res.results]
    return np.concatenate(outs, axis=0)


# revision 17
# speedup vs baseline: 2.2968x; 1.1706x over previous
"""Trainium2 Bass kernel for nn_Encoder_Postnet (B=16, T=8192, TP=512, E=256).

Exact algebra:
    idx  = aligner_indices(align_phone, text_phone)     # host scan (sequential int walk)
    out  = enc2[b, idx] + PEW[t] + pitch[b,t]*Wp + beats[b,t]*EBd
where
    enc2 = encoder_out @ (I + W_pos)                    # device PE, f32
    PEW  = pe @ W_pos + b_pos + b_pitch + emb_beats[0]  # host constant table
    Wp   = W_pitch[0],  EBd = emb_beats[1] - emb_beats[0]

Sharding: pure data parallel, 2 batches per core across 8 cores.

The frame gather enc2[idx] exploits idx monotonicity: a 128-frame group's rows
span few enc2 rows, so the gather becomes one matmul per group on the
(otherwise idle) TensorE against a 64-aligned "window" of enc2: the device
keeps enc2win[k] = [enc2 rows 64k..64k+126; Wp; EBd] (128 partitions), and the
host builds lhsT columns: rows 0-125 one-hot selecting the window row per
frame, rows 126-127 carrying pitch/beats so the rank-2 aux term rides in the
SAME matmul. PSUM accumulates; one DVE tensor_tensor adds the resident PEW
table and writes fp16 output. Arbitrary idx is handled by covering a group
with multiple windows (extra accumulating matmuls); SPMD uniformity by taking
the cross-core union of window entries (unused entries have all-zero lhsT
columns).
"""
import numpy as np

import concourse.bacc as bacc
import concourse.bass as bass
import concourse.mybir as mybir
import concourse.tile as tile
from concourse.bass_utils import run_bass_kernel_spmd

# ---- problem constants (hardcoded per harness contract) ----
B, T, TP, E = 16, 8192, 512, 256
NCORES = 8
BPC = B // NCORES            # batches per core = 2
ROWS = BPC * TP              # enc2 rows per core = 1024
NBLK = ROWS // 128           # 128-row blocks of enc2 = 8
NWIN = ROWS // 64            # 64-aligned windows = 16
CH = 1024                    # frames per chunk
NCH = T // CH                # chunks per batch = 8
NG = CH // 128               # 128-frame groups per chunk = 8
NCHUNK = BPC * NCH           # chunks per core = 16
WROWS = 126                  # usable enc2 rows per window (126/127 = Wp/EBd)

F32 = mybir.dt.float32
FP16 = mybir.dt.float16

_PROGRAM_CACHE: dict = {}


# ---------------- host-side pieces ----------------

def aligner_idx_host(align_phone: np.ndarray, text_phone: np.ndarray) -> np.ndarray:
    """Exact numpy equivalent of the reference aligner_indices scan."""
    b, t = align_phone.shape
    tp_last = text_phone.shape[1] - 1
    idx = np.zeros((b, t), dtype=np.int32)
    ind = np.zeros(b, dtype=np.int32)
    before = text_phone[:, 0].copy()
    barange = np.arange(b)
    for j in range(1, t):
        a = align_phone[:, j]
        same = a == before
        ind = np.minimum(np.where(same, ind, ind + 1), tp_last)
        before = np.where(same, before, text_phone[barange, ind])
        idx[:, j] = ind
    return idx


def sinusoid_pe_host(length, dim):
    pos = np.arange(length, dtype=np.float32)[:, None]
    div = np.exp(np.arange(0, dim, 2, dtype=np.float32) * (-(np.log(10000.0) / dim)))
    ang = pos * div
    pe = np.zeros((length, dim), np.float32)
    pe[:, 0::2] = np.sin(ang)
    pe[:, 1::2] = np.cos(ang)
    return pe


def windows_for_group(gi: np.ndarray) -> list:
    """Minimal 64-aligned 126-row windows covering the rows in gi (sorted)."""
    rows = np.unique(gi)
    wins = []
    i = 0
    while i < len(rows):
        k = int(rows[i]) // 64
        wins.append(k)
        top = 64 * k + WROWS  # rows [64k, 64k+126) covered
        while i < len(rows) and rows[i] < top:
            i += 1
    return wins


def group_windows(idx_rows: np.ndarray):
    """per chunk per group: list of window ids for this core."""
    out = []
    for lb in range(BPC):
        for c in range(NCH):
            chunk = []
            for g in range(NG):
                f0 = c * CH + g * 128
                chunk.append(windows_for_group(idx_rows[lb, f0:f0 + 128]))
            out.append(chunk)
    return out


# ---------------- device program ----------------

def build_program(canon_plan, ncols_total) -> bass.Bass:
    """canon_plan[ci][g] = list of (coloff, win_k)."""
    nc = bacc.Bacc("TRN2", num_devices=NCORES, debug=False, enable_asserts=False)

    enc = nc.dram_tensor("enc", [ROWS, E], F32, kind="ExternalInput")
    w2 = nc.dram_tensor("w2", [E, E], F32, kind="ExternalInput")
    ident = nc.dram_tensor("ident", [128, 128], F32, kind="ExternalInput")
    w3rep = nc.dram_tensor("w3rep", [2, NWIN, E], FP16, kind="ExternalInput")
    pew = nc.dram_tensor("pew", [T, E], FP16, kind="ExternalInput")
    oh = nc.dram_tensor("oh", [128, ncols_total], FP16, kind="ExternalInput")
    out = nc.dram_tensor("out", [BPC * T, E], FP16, kind="ExternalOutput")

    with tile.TileContext(nc) as tc:
        with (
            tc.tile_pool(name="const", bufs=1) as cpool,
            tc.tile_pool(name="outp", bufs=3) as opool,
        ):
            # ---- constants (spread issue across HWDGE engines) ----
            pew_sb = cpool.tile([128, T // 128, E], FP16, tag="pew")
            nc.sync.dma_start(pew_sb[:], pew.ap().rearrange("(c p) e -> p c e", p=128))
            w2_sb = cpool.tile([128, 2, E], F32, tag="w2")
            nc.scalar.dma_start(w2_sb[:], w2.ap().rearrange("(k p) e -> p k e", p=128))
            ident_sb = cpool.tile([128, 128], F32, tag="ident")
            nc.scalar.dma_start(ident_sb[:], ident.ap())
            oh_sb = cpool.tile([128, ncols_total], FP16, tag="oh")
            nc.scalar.dma_start(oh_sb[:], oh.ap())

            # ---- prologue: enc2 = enc @ (I + W_pos), fp16, then windows ----
            enc_sb = cpool.tile([128, NBLK, E], F32, tag="enc")
            nc.gpsimd.dma_start(enc_sb[:], enc.ap().rearrange("(r p) e -> p r e", p=128))
            encT_sb = cpool.tile([128, 2 * NBLK, 128], F32, tag="encT")
            enc2_sb = cpool.tile([128, NBLK, E], FP16, tag="enc2")
            win_sb = cpool.tile([128, NWIN, E], FP16, tag="win")
            nc.gpsimd.memset(win_sb[:], 0.0)
            with tc.tile_pool(name="psum_pro", bufs=4, space="PSUM") as ppro:
                for rt in range(NBLK):
                    for k in range(2):
                        pt = ppro.tile([128, 128], F32, tag="ptr")
                        nc.tensor.transpose(
                            out=pt[:],
                            in_=enc_sb[:, rt, k * 128:(k + 1) * 128],
                            identity=ident_sb[:],
                        )
                        nc.vector.tensor_copy(
                            out=encT_sb[:, k * NBLK + rt, :], in_=pt[:]
                        )
                for rt in range(NBLK):
                    pe2 = ppro.tile([128, E], F32, tag="pe2")
                    nc.tensor.matmul(
                        out=pe2[:], lhsT=encT_sb[:, rt, :], rhs=w2_sb[:, 0, :],
                        start=True, stop=False,
                    )
                    nc.tensor.matmul(
                        out=pe2[:], lhsT=encT_sb[:, NBLK + rt, :],
                        rhs=w2_sb[:, 1, :], start=False, stop=True,
                    )
                    nc.vector.tensor_copy(out=enc2_sb[:, rt, :], in_=pe2[:])
            # windows via 4 strided DMAs:
            # even k: rows [128(k/2), +126) = aligned block copy
            nc.scalar.dma_start(win_sb[0:126, 0:NWIN:2, :], enc2_sb[0:126, :, :])
            # odd k piece 1: rows [64k, 64k+64) = block k//2 partitions 64..128
            nc.scalar.dma_start(win_sb[0:64, 1:NWIN:2, :], enc2_sb[64:128, :, :])
            # odd k piece 2: rows [64k+64, 64k+126) = block k//2+1 partitions 0..62
            nc.scalar.dma_start(
                win_sb[64:126, 1:NWIN - 2:2, :], enc2_sb[0:62, 1:NBLK, :]
            )
            # constant rows 126/127 = Wp, EBd for every window
            nc.scalar.dma_start(win_sb[126:128, :, :], w3rep.ap())

            # ---- main loop ----
            with tc.tile_pool(name="psum_main", bufs=2, space="PSUM") as pmain:
                for ci in range(NCHUNK):
                    lb, c = divmod(ci, NCH)
                    ps = pmain.tile([128, NG, E], F32, tag="ps")
                    for g in range(NG):
                        entries = canon_plan[ci][g]
                        n = len(entries)
                        for j, (coloff, k) in enumerate(entries):
                            nc.tensor.matmul(
                                out=ps[:, g, :],
                                lhsT=oh_sb[:, coloff:coloff + 128],
                                rhs=win_sb[:, k, :],
                                start=(j == 0), stop=(j == n - 1),
                            )
                    o = opool.tile([128, NG, E], FP16, tag="o")
                    nc.vector.tensor_tensor(
                        out=o[:], in0=ps[:],
                        in1=pew_sb[:, c * NG:(c + 1) * NG, :],
                        op=mybir.AluOpType.add,
                    )
                    base = lb * T + c * CH
                    out_eng = nc.sync if ci % 2 == 0 else nc.scalar
                    out_eng.dma_start(
                        out.ap()[base:base + CH, :].rearrange(
                            "(cc p) e -> p cc e", p=128
                        ),
                        o[:],
                    )
    nc.compile()
    return nc


# ---------------- host orchestration ----------------

def make_in_maps(encoder_out, align_phone, text_phone, pitch, beats,
                 W_pitch, b_pitch, W_pos, b_pos, emb_beats):
    idx = aligner_idx_host(np.asarray(align_phone), np.asarray(text_phone))  # [B, T]

    pe = sinusoid_pe_host(T, E)
    pew = (pe @ np.asarray(W_pos) + np.asarray(b_pos) + np.asarray(b_pitch)
           + np.asarray(emb_beats)[0]).astype(np.float32)
    w2 = (np.eye(E, dtype=np.float32) + np.asarray(W_pos)).astype(np.float32)
    ident = np.eye(128, dtype=np.float32)
    wp = np.asarray(W_pitch)[0].astype(np.float32)
    ebd = (np.asarray(emb_beats)[1] - np.asarray(emb_beats)[0]).astype(np.float32)
    w3 = np.stack([wp, ebd]).astype(np.float16)  # [2, E]
    w3rep = np.broadcast_to(w3[:, None, :], (2, NWIN, E)).copy()

    enc = np.ascontiguousarray(np.asarray(encoder_out), dtype=np.float32)  # [B, TP, E]
    pitch2 = np.asarray(pitch)[:, :, 0].astype(np.float32)
    beats2 = np.asarray(beats)[:, :, 0].astype(np.float32)

    idx_rows_all = []
    wins_all = []
    for core in range(NCORES):
        bs = slice(core * BPC, (core + 1) * BPC)
        idx_rows = idx[bs] + (np.arange(BPC)[:, None] * TP)
        idx_rows_all.append(idx_rows)
        wins_all.append(group_windows(idx_rows))

    # canonical plan: per (chunk, group) union of window ids across cores
    canon_plan = []
    off = 0
    for ci in range(NCHUNK):
        chunk_plan = []
        for g in range(NG):
            ks = sorted({k for core in range(NCORES) for k in wins_all[core][ci][g]})
            entries = []
            for k in ks:
                entries.append((off, k))
                off += 128
            chunk_plan.append(entries)
        canon_plan.append(chunk_plan)
    ncols_total = off

    per_core = []
    m = np.arange(128)
    for core in range(NCORES):
        bs = slice(core * BPC, (core + 1) * BPC)
        idx_rows = idx_rows_all[core]
        onehot = np.zeros((128, ncols_total), dtype=np.float16)
        for ci in range(NCHUNK):
            lb, c = divmod(ci, NCH)
            for g in range(NG):
                f0 = c * CH + g * 128
                gi = idx_rows[lb, f0:f0 + 128]
                my_wins = wins_all[core][ci][g]
                entries = canon_plan[ci][g]
                # row -> my window (first of my windows covering it)
                assigned = np.full(128, -1, dtype=np.int64)
                for k in my_wins:
                    in_win = (gi >= 64 * k) & (gi < 64 * k + WROWS) & (assigned < 0)
                    assigned[in_win] = k
                aux_done = False
                for (coloff, k) in entries:
                    if k not in my_wins:
                        continue
                    sel = assigned == k
                    onehot[gi[sel] - 64 * k, coloff + m[sel]] = 1.0
                    if not aux_done:
                        fr = slice(c * CH + g * 128, c * CH + g * 128 + 128)
                        onehot[126, coloff:coloff + 128] = pitch2[core * BPC + lb, fr]
                        onehot[127, coloff:coloff + 128] = beats2[core * BPC + lb, fr]
                        aux_done = True
        per_core.append({
            "enc": enc[bs].reshape(ROWS, E),
            "w2": w2,
            "ident": ident,
            "w3rep": w3rep,
            "pew": pew.astype(np.float16),
            "oh": onehot,
        })

    return per_core, canon_plan, ncols_total


def get_program(canon_plan, ncols_total) -> bass.Bass:
    key = (tuple(tuple(tuple(e) for e in cg) for cg in canon_plan), ncols_total)
    if key not in _PROGRAM_CACHE:
        _PROGRAM_CACHE[key] = build_program(canon_plan, ncols_total)
    return _PROGRAM_CACHE[key]


def kernel(**inputs) -> np.ndarray:
    in_maps, canon_plan, ncols_total = make_in_maps(**inputs)
    nc = get_program(canon_plan, ncols_total)
    res = run_bass_kernel_spmd(nc, in_maps, core_ids=list(range(NCORES)))
    outs = [r["out"].astype(np.float32).reshape(BPC, T, E) for r in res.results]
    return np.concatenate(outs, axis=0)
